# revision 39
# baseline (speedup 1.0000x reference)
"""LookupConv2d Trainium2 kernel.

Math: out = conv2d(x, W), W[o] = sum_s coeff[o,s] * dictionary[idx[o,s]].
Factorization: W = M @ D where M[o,d] = sum_{s: idx[o,s]=d} coeff[o,s] is a
(512, 100) scatter of the coefficients.  Then
    out = M @ conv2d(x, dictionary)
i.e. a 100-channel conv (23 GFLOP) followed by a 1x1 512x100 mix (5 GFLOP)
instead of a 512-channel conv (118 GFLOP) -- 4.2x fewer FLOPs.

Precision: fp16 operands (11-bit mantissa) with fp32 PSUM accumulation.
TensorE streams fp16 at 1 row/cycle (same rate as bf16/f32r, 4x fp32 mode).
Per-operand rounding is ~2^-12 relative; over the 2304-term conv reduction
the output error stays ~4e-4 relative -- far inside the 2e-2 gate -- while
using a third of the PE cycles of the fp32-class split-matmul scheme and
half the input DMA bytes.

Schedule: row-tiles of 9,9,9,9,9,8,3 rows per image (the 3-row tail keeps
the end-of-kernel mix+copy+DMA chain short).  PE is pre-warmed with dummy
matmuls (reading a not-yet-written x region, results to an unread PSUM
bank) so the clock is fully ramped and the tensor engine never idles
while the first weight/input DMAs are in flight.  Each tile's mix is
split into ob-pair halves interleaved into the next tile's conv, so
output copies/DMAs start half a tile early and stream out continuously.
Output is staged in SBUF as fp16 (host upcasts to fp32; ~1.4e-4 extra
RMS rounding) which halves the output DMA bytes and keeps the kernel
compute-bound.

Sharding: data-parallel over batch N=16 -> 2 images per core on 8 cores.
dictionary (as [128, 1800] fp16 tap matrices) and M^T are replicated.
"""

import numpy as np

N_CORES = 8
IMGS_PER_CORE = 2
CIN = 256
COUT = 512
NDICT = 100
H = W = 56
HP = WP = 58  # padded
S = 3  # lookup sparsity
HW = H * W  # 3136

# (h0, rows) conv tiles per image; free dim = rows*W <= 504 (PSUM bank cap)
TILES = [(0, 9), (9, 9), (18, 9), (27, 9), (36, 9), (45, 8), (53, 3)]
# padded-input row chunks; tile t only needs chunks 0..ceil
ROW_CHUNKS = [(0, 11), (11, 9), (20, 9), (29, 9), (38, 9), (47, 11)]
MAX_FREE = 9 * W  # 504

TRACE = False  # set by test.py to get a profile
_LAST_RESULTS = {}  # test.py reads exec_time_ns from here


def _build_program():
    import concourse.bacc as bacc
    import concourse.mybir as mybir
    import concourse.tile as tile

    f32 = mybir.dt.float32
    f16 = mybir.dt.float16

    nc = bacc.Bacc("TRN2", target_bir_lowering=False, debug=False)

    x_d = nc.dram_tensor("x", (IMGS_PER_CORE, CIN, HP, WP), f16,
                         kind="ExternalInput")
    w_d = nc.dram_tensor("w", (128, 2 * 9 * NDICT), f16, kind="ExternalInput")
    m_d = nc.dram_tensor("m", (NDICT, COUT), f16, kind="ExternalInput")
    out_d = nc.dram_tensor("out", (IMGS_PER_CORE, COUT, H, W), f16,
                           kind="ExternalOutput")

    with tile.TileContext(nc) as tc:
        with (
            tc.tile_pool(name="consts", bufs=1) as consts,
            tc.tile_pool(name="xpool", bufs=1) as xpool,
            tc.tile_pool(name="ypool", bufs=4) as ypool,
            tc.tile_pool(name="opool", bufs=2) as opool,
            tc.tile_pool(name="psum_y", bufs=2, space="PSUM") as psum_y_pool,
            tc.tile_pool(name="psum_o", bufs=6, space="PSUM") as psum_o_pool,
        ):
            # --- constants + input tiles
            w_sb = consts.tile([128, 2 * 9 * NDICT], f16)
            m_sb = consts.tile([NDICT, COUT], f16)
            x_sb = xpool.tile([128, IMGS_PER_CORE, 2, HP, WP], f16, tag="x_sb")
            x_v = x_d.rearrange("i (b c) h w -> c i b h w", c=128)

            # --- PE pre-warm: dummy matmuls keep the PE clock ramping while
            # the first weight/input DMAs are in flight.  They read the img1
            # region of x_sb, which is written only much later (WAR dep), so
            # they start immediately; results go to a scratch PSUM bank that
            # is never read.
            pwarm = psum_o_pool.tile([128, MAX_FREE], f32, tag="po",
                                     name="pwarm")
            for _ in range(33):
                nc.tensor.matmul(pwarm[:58, :116], x_sb[:, 1, 0, 0, 0:58],
                                 x_sb[:, 1, 0, 1:3, :],
                                 start=True, stop=True)

            def dma_x(img, cb, chunk):
                r0, nr = ROW_CHUNKS[chunk]
                nc.sync.dma_start(x_sb[:, img, cb, r0:r0 + nr, :],
                                  x_v[:, img, cb, r0:r0 + nr, :])

            nc.sync.dma_start(w_sb[:, :900], w_d[:, :900])
            dma_x(0, 0, 0)
            nc.sync.dma_start(w_sb[:, 900:], w_d[:, 900:])
            dma_x(0, 1, 0)
            nc.sync.dma_start(m_sb[:], m_d[:])
            for img in range(IMGS_PER_CORE):
                for chunk in range(1 if img == 0 else 0, len(ROW_CHUNKS)):
                    for cb in range(2):
                        if img == 0 and chunk == 0 and cb == 0:
                            continue
                        dma_x(img, cb, chunk)

            out_v = out_d.rearrange("i (b o) h w -> i b o (h w)", o=128)
            # same tensor viewed [img, o, b, hw] so a [128, 4, cols] SBUF
            # tile maps element-for-element in the tail DMA
            out_vt = out_d.rearrange("i (b o) h w -> i o b (h w)", o=128)

            def emit_conv_taps(py, img, ti_idx, cb, t0, t1):
                h0, rows = TILES[ti_idx]
                fd = rows * W
                for tt in range(t0, t1):
                    ti, tj = divmod(tt, 3)
                    k = cb * 9 + tt
                    tap = slice(k * NDICT, (k + 1) * NDICT)
                    rh = (slice(None), img, cb,
                          slice(h0 + ti, h0 + ti + rows),
                          slice(tj, tj + W))
                    nc.tensor.matmul(
                        py[:, :fd], w_sb[:, tap], x_sb[rh],
                        start=(k == 0), stop=(k == 17))

            o_accs = {}

            def emit_y_copy(py, img, ti_idx):
                _, rows = TILES[ti_idx]
                fd = rows * W
                if ti_idx == 0:
                    o_accs[img] = opool.tile([128, 4, HW], f16, tag="oacc",
                                             name=f"oacc{img}")
                y16 = ypool.tile([NDICT, MAX_FREE], f16, tag="y16")
                nc.scalar.copy(y16[:, :fd], py[:, :fd])
                return y16

            def emit_mix_half(y16, img, ti_idx, half):
                h0, rows = TILES[ti_idx]
                fd = rows * W
                c0 = h0 * W
                o_acc = o_accs[img]
                if fd <= 256:
                    # both obs of this half fit in one PSUM bank: 2 matmuls,
                    # then a single copy (fewer fixed costs on the end chain)
                    po = psum_o_pool.tile([128, MAX_FREE], f32, tag="po")
                    po2 = po.rearrange("p (b f) -> p b f", b=2)
                    for i, ob in enumerate((2 * half, 2 * half + 1)):
                        obs = slice(ob * 128, (ob + 1) * 128)
                        nc.tensor.matmul(po2[:, i, :fd], m_sb[:, obs],
                                         y16[:, :fd], start=True, stop=True)
                    dst = o_acc[:, 2 * half:2 * half + 2, c0:c0 + fd]
                    if half == 0:
                        nc.vector.tensor_copy(dst, po2[:, :, :fd])
                    else:
                        nc.scalar.copy(dst, po2[:, :, :fd])
                else:
                    for ob in (2 * half, 2 * half + 1):
                        obs = slice(ob * 128, (ob + 1) * 128)
                        po = psum_o_pool.tile([128, MAX_FREE], f32, tag="po")
                        nc.tensor.matmul(po[:, :fd], m_sb[:, obs],
                                         y16[:, :fd], start=True, stop=True)
                        dst = o_acc[:, ob, c0:c0 + fd]
                        if ob % 2 == 0:
                            nc.vector.tensor_copy(dst, po[:, :fd])
                        else:
                            nc.scalar.copy(dst, po[:, :fd])
                # per-ob-pair DMA: output streams out as soon as each half of
                # the tile's mix lands (no end-of-kernel backlog).  Small
                # tail tiles go as a single all-ob DMA (HWDGE slot latency
                # dominates their transfer time).
                if fd >= 300:
                    nc.sync.dma_start(
                        out_vt[img, :, 2 * half:2 * half + 2, c0:c0 + fd],
                        o_acc[:, 2 * half:2 * half + 2, c0:c0 + fd])
                elif half == 1:
                    nc.sync.dma_start(out_vt[img, :, :, c0:c0 + fd],
                                      o_acc[:, :, c0:c0 + fd])

            # software-pipeline by one tile, with tile i-1's mix matmuls
            # interleaved into the middle of tile i's conv so its output
            # copies/DMA start ~half a tile earlier
            prev = None
            for img in range(IMGS_PER_CORE):
                for t in range(len(TILES)):
                    py = psum_y_pool.tile([NDICT, MAX_FREE], f32, tag="py",
                                          name=f"py{img}_{t}")
                    emit_conv_taps(py, img, t, 0, 0, 5)
                    if prev is not None:
                        emit_mix_half(*prev, 0)
                    emit_conv_taps(py, img, t, 0, 5, 9)
                    emit_conv_taps(py, img, t, 1, 0, 9)
                    y16 = emit_y_copy(py, img, t)
                    if prev is not None:
                        emit_mix_half(*prev, 1)
                    prev = (y16, img, t)
            emit_mix_half(*prev, 0)
            emit_mix_half(*prev, 1)

    nc.compile()
    return nc


_NC_CACHE = None


def kernel(x, dictionary, lookup_indices, lookup_coefficients):
    global _NC_CACHE
    from concourse import bass_utils

    x = np.asarray(x, dtype=np.float32)
    dictionary = np.asarray(dictionary, dtype=np.float32)
    idx = np.asarray(lookup_indices).astype(np.int64)
    coef = np.asarray(lookup_coefficients, dtype=np.float32)

    # M^T[d, o] = sum_s coeff[o, s] * [idx[o, s] == d]
    mt = np.zeros((NDICT, COUT), np.float32)
    np.add.at(mt, (idx.reshape(-1),
                   np.repeat(np.arange(COUT), S)), coef.reshape(-1))

    # wt[c_in_block, (cblk, ti, tj, d)] = dictionary[d, cblk*128+c, ti, tj]
    wt = np.ascontiguousarray(
        dictionary.reshape(NDICT, 2, 128, 3, 3).transpose(2, 1, 3, 4, 0)
    ).reshape(128, 2 * 9 * NDICT)

    xp = np.pad(x, ((0, 0), (0, 0), (1, 1), (1, 1)))
    xp = np.ascontiguousarray(
        xp.reshape(N_CORES, IMGS_PER_CORE, CIN, HP, WP)).astype(np.float16)
    wt16 = wt.astype(np.float16)
    mt16 = mt.astype(np.float16)

    if _NC_CACHE is None:
        _NC_CACHE = _build_program()
    nc = _NC_CACHE

    in_maps = [{"x": xp[i], "w": wt16, "m": mt16} for i in range(N_CORES)]
    try:
        res = bass_utils.run_bass_kernel_spmd(
            nc, in_maps, core_ids=list(range(N_CORES)), trace=TRACE)
    except ModuleNotFoundError:
        # no axon NTFF profile hook in this environment
        res = bass_utils.run_bass_kernel_spmd(
            nc, in_maps, core_ids=list(range(N_CORES)), trace=False)
    _LAST_RESULTS["res"] = res

    out = np.concatenate([r["out"] for r in res.results], axis=0)
    return out.reshape(16, COUT, H, W).astype(np.float32)


# revision 40
# speedup vs baseline: 1.0043x; 1.0043x over previous
"""LookupConv2d Trainium2 kernel.

Math: out = conv2d(x, W), W[o] = sum_s coeff[o,s] * dictionary[idx[o,s]].
Factorization: W = M @ D where M[o,d] = sum_{s: idx[o,s]=d} coeff[o,s] is a
(512, 100) scatter of the coefficients.  Then
    out = M @ conv2d(x, dictionary)
i.e. a 100-channel conv (23 GFLOP) followed by a 1x1 512x100 mix (5 GFLOP)
instead of a 512-channel conv (118 GFLOP) -- 4.2x fewer FLOPs.

Precision: fp16 operands (11-bit mantissa) with fp32 PSUM accumulation.
TensorE streams fp16 at 1 row/cycle (same rate as bf16/f32r, 4x fp32 mode).
Per-operand rounding is ~2^-12 relative; over the 2304-term conv reduction
the output error stays ~4e-4 relative -- far inside the 2e-2 gate -- while
using a third of the PE cycles of the fp32-class split-matmul scheme and
half the input DMA bytes.

Schedule: row-tiles of 9,9,9,9,9,8,3 rows per image (the 3-row tail keeps
the end-of-kernel mix+copy+DMA chain short).  PE is pre-warmed with dummy
matmuls (reading a not-yet-written x region, results to an unread PSUM
bank) so the clock is fully ramped and the tensor engine never idles
while the first weight/input DMAs are in flight.  Each tile's mix is
split into ob-pair halves interleaved into the next tile's conv, so
output copies/DMAs start half a tile early and stream out continuously.
Output is staged in SBUF as fp16 (host upcasts to fp32; ~1.4e-4 extra
RMS rounding) which halves the output DMA bytes and keeps the kernel
compute-bound.

Sharding: data-parallel over batch N=16 -> 2 images per core on 8 cores.
dictionary (as [128, 1800] fp16 tap matrices) and M^T are replicated.
"""

import numpy as np

N_CORES = 8
IMGS_PER_CORE = 2
CIN = 256
COUT = 512
NDICT = 100
H = W = 56
HP = WP = 58  # padded
S = 3  # lookup sparsity
HW = H * W  # 3136

# (h0, rows) conv tiles per image; free dim = rows*W <= 504 (PSUM bank cap)
TILES = [(0, 9), (9, 9), (18, 9), (27, 9), (36, 9), (45, 8), (53, 3)]
# padded-input row chunks; tile t only needs chunks 0..ceil
ROW_CHUNKS = [(0, 11), (11, 9), (20, 9), (29, 9), (38, 9), (47, 11)]
MAX_FREE = 9 * W  # 504

TRACE = False  # set by test.py to get a profile
_LAST_RESULTS = {}  # test.py reads exec_time_ns from here


def _build_program():
    import concourse.bacc as bacc
    import concourse.mybir as mybir
    import concourse.tile as tile

    f32 = mybir.dt.float32
    f16 = mybir.dt.float16

    nc = bacc.Bacc("TRN2", target_bir_lowering=False, debug=False)

    x_d = nc.dram_tensor("x", (IMGS_PER_CORE, CIN, HP, WP), f16,
                         kind="ExternalInput")
    w_d = nc.dram_tensor("w", (128, 2 * 9 * NDICT), f16, kind="ExternalInput")
    m_d = nc.dram_tensor("m", (NDICT, COUT), f16, kind="ExternalInput")
    out_d = nc.dram_tensor("out", (IMGS_PER_CORE, COUT, H, W), f16,
                           kind="ExternalOutput")

    with tile.TileContext(nc) as tc:
        with (
            tc.tile_pool(name="consts", bufs=1) as consts,
            tc.tile_pool(name="xpool", bufs=1) as xpool,
            tc.tile_pool(name="ypool", bufs=4) as ypool,
            tc.tile_pool(name="opool", bufs=2) as opool,
            tc.tile_pool(name="psum_y", bufs=2, space="PSUM") as psum_y_pool,
            tc.tile_pool(name="psum_o", bufs=6, space="PSUM") as psum_o_pool,
        ):
            # --- constants + input tiles
            w_sb = consts.tile([128, 2 * 9 * NDICT], f16)
            m_sb = consts.tile([NDICT, COUT], f16)
            x_sb = xpool.tile([128, IMGS_PER_CORE, 2, HP, WP], f16, tag="x_sb")
            x_v = x_d.rearrange("i (b c) h w -> c i b h w", c=128)

            # --- PE pre-warm: dummy matmuls keep the PE clock ramping while
            # the first weight/input DMAs are in flight.  They read the img1
            # region of x_sb, which is written only much later (WAR dep), so
            # they start immediately; results go to a scratch PSUM bank that
            # is never read.
            pwarm = psum_o_pool.tile([128, MAX_FREE], f32, tag="po",
                                     name="pwarm")
            for _ in range(33):
                nc.tensor.matmul(pwarm[:58, :116], x_sb[:, 1, 0, 0, 0:58],
                                 x_sb[:, 1, 0, 1:3, :],
                                 start=True, stop=True)

            def dma_x(img, cb, chunk):
                r0, nr = ROW_CHUNKS[chunk]
                nc.sync.dma_start(x_sb[:, img, cb, r0:r0 + nr, :],
                                  x_v[:, img, cb, r0:r0 + nr, :])

            nc.sync.dma_start(w_sb[:, :900], w_d[:, :900])
            dma_x(0, 0, 0)
            nc.sync.dma_start(w_sb[:, 900:], w_d[:, 900:])
            dma_x(0, 1, 0)
            nc.sync.dma_start(m_sb[:], m_d[:])
            for img in range(IMGS_PER_CORE):
                for chunk in range(1 if img == 0 else 0, len(ROW_CHUNKS)):
                    for cb in range(2):
                        if img == 0 and chunk == 0 and cb == 0:
                            continue
                        dma_x(img, cb, chunk)

            out_v = out_d.rearrange("i (b o) h w -> i b o (h w)", o=128)
            # same tensor viewed [img, o, b, hw] so a [128, 4, cols] SBUF
            # tile maps element-for-element in the tail DMA
            out_vt = out_d.rearrange("i (b o) h w -> i o b (h w)", o=128)

            def emit_conv_taps(py, img, ti_idx, cb, t0, t1):
                h0, rows = TILES[ti_idx]
                fd = rows * W
                for tt in range(t0, t1):
                    ti, tj = divmod(tt, 3)
                    k = cb * 9 + tt
                    tap = slice(k * NDICT, (k + 1) * NDICT)
                    rh = (slice(None), img, cb,
                          slice(h0 + ti, h0 + ti + rows),
                          slice(tj, tj + W))
                    nc.tensor.matmul(
                        py[:, :fd], w_sb[:, tap], x_sb[rh],
                        start=(k == 0), stop=(k == 17))

            o_accs = {}

            def emit_y_copy(py, img, ti_idx):
                _, rows = TILES[ti_idx]
                fd = rows * W
                if ti_idx == 0:
                    o_accs[img] = opool.tile([128, 4, HW], f16, tag="oacc",
                                             name=f"oacc{img}")
                y16 = ypool.tile([NDICT, MAX_FREE], f16, tag="y16")
                nc.scalar.copy(y16[:, :fd], py[:, :fd])
                return y16

            def emit_mix_half(y16, img, ti_idx, half):
                h0, rows = TILES[ti_idx]
                fd = rows * W
                c0 = h0 * W
                o_acc = o_accs[img]
                if fd <= 256:
                    # both obs of this half fit in one PSUM bank: 2 matmuls,
                    # then a single copy (fewer fixed costs on the end chain)
                    po = psum_o_pool.tile([128, MAX_FREE], f32, tag="po")
                    po2 = po.rearrange("p (b f) -> p b f", b=2)
                    for i, ob in enumerate((2 * half, 2 * half + 1)):
                        obs = slice(ob * 128, (ob + 1) * 128)
                        nc.tensor.matmul(po2[:, i, :fd], m_sb[:, obs],
                                         y16[:, :fd], start=True, stop=True)
                    dst = o_acc[:, 2 * half:2 * half + 2, c0:c0 + fd]
                    if half == 0:
                        nc.vector.tensor_copy(dst, po2[:, :, :fd])
                    else:
                        nc.scalar.copy(dst, po2[:, :, :fd])
                else:
                    for ob in (2 * half, 2 * half + 1):
                        obs = slice(ob * 128, (ob + 1) * 128)
                        po = psum_o_pool.tile([128, MAX_FREE], f32, tag="po")
                        nc.tensor.matmul(po[:, :fd], m_sb[:, obs],
                                         y16[:, :fd], start=True, stop=True)
                        dst = o_acc[:, ob, c0:c0 + fd]
                        if ob % 2 == 0:
                            nc.vector.tensor_copy(dst, po[:, :fd])
                        else:
                            nc.scalar.copy(dst, po[:, :fd])
                # per-ob-pair DMA: output streams out as soon as each half of
                # the tile's mix lands (no end-of-kernel backlog).  Small
                # tail tiles go as a single all-ob DMA (HWDGE slot latency
                # dominates their transfer time).
                if fd >= 300:
                    nc.sync.dma_start(
                        out_vt[img, :, 2 * half:2 * half + 2, c0:c0 + fd],
                        o_acc[:, 2 * half:2 * half + 2, c0:c0 + fd])
                elif half == 1:
                    nc.sync.dma_start(out_vt[img, :, :, c0:c0 + fd],
                                      o_acc[:, :, c0:c0 + fd])

            # software-pipeline by one tile, with tile i-1's mix matmuls
            # interleaved into the middle of tile i's conv so its output
            # copies/DMA start ~half a tile earlier
            prev = None
            for img in range(IMGS_PER_CORE):
                for t in range(len(TILES)):
                    py = psum_y_pool.tile([NDICT, MAX_FREE], f32, tag="py",
                                          name=f"py{img}_{t}")
                    emit_conv_taps(py, img, t, 0, 0, 9)
                    if prev is not None:
                        emit_mix_half(*prev, 0)
                    emit_conv_taps(py, img, t, 1, 0, 9)
                    y16 = emit_y_copy(py, img, t)
                    if prev is not None:
                        emit_mix_half(*prev, 1)
                    prev = (y16, img, t)
            emit_mix_half(*prev, 0)
            emit_mix_half(*prev, 1)

    nc.compile()
    return nc


_NC_CACHE = None


def kernel(x, dictionary, lookup_indices, lookup_coefficients):
    global _NC_CACHE
    from concourse import bass_utils

    x = np.asarray(x, dtype=np.float32)
    dictionary = np.asarray(dictionary, dtype=np.float32)
    idx = np.asarray(lookup_indices).astype(np.int64)
    coef = np.asarray(lookup_coefficients, dtype=np.float32)

    # M^T[d, o] = sum_s coeff[o, s] * [idx[o, s] == d]
    mt = np.zeros((NDICT, COUT), np.float32)
    np.add.at(mt, (idx.reshape(-1),
                   np.repeat(np.arange(COUT), S)), coef.reshape(-1))

    # wt[c_in_block, (cblk, ti, tj, d)] = dictionary[d, cblk*128+c, ti, tj]
    wt = np.ascontiguousarray(
        dictionary.reshape(NDICT, 2, 128, 3, 3).transpose(2, 1, 3, 4, 0)
    ).reshape(128, 2 * 9 * NDICT)

    xp = np.pad(x, ((0, 0), (0, 0), (1, 1), (1, 1)))
    xp = np.ascontiguousarray(
        xp.reshape(N_CORES, IMGS_PER_CORE, CIN, HP, WP)).astype(np.float16)
    wt16 = wt.astype(np.float16)
    mt16 = mt.astype(np.float16)

    if _NC_CACHE is None:
        _NC_CACHE = _build_program()
    nc = _NC_CACHE

    in_maps = [{"x": xp[i], "w": wt16, "m": mt16} for i in range(N_CORES)]
    try:
        res = bass_utils.run_bass_kernel_spmd(
            nc, in_maps, core_ids=list(range(N_CORES)), trace=TRACE)
    except ModuleNotFoundError:
        # no axon NTFF profile hook in this environment
        res = bass_utils.run_bass_kernel_spmd(
            nc, in_maps, core_ids=list(range(N_CORES)), trace=False)
    _LAST_RESULTS["res"] = res

    out = np.concatenate([r["out"] for r in res.results], axis=0)
    return out.reshape(16, COUT, H, W).astype(np.float32)


# revision 41
# speedup vs baseline: 1.0123x; 1.0079x over previous
"""LookupConv2d Trainium2 kernel.

Math: out = conv2d(x, W), W[o] = sum_s coeff[o,s] * dictionary[idx[o,s]].
Factorization: W = M @ D where M[o,d] = sum_{s: idx[o,s]=d} coeff[o,s] is a
(512, 100) scatter of the coefficients.  Then
    out = M @ conv2d(x, dictionary)
i.e. a 100-channel conv (23 GFLOP) followed by a 1x1 512x100 mix (5 GFLOP)
instead of a 512-channel conv (118 GFLOP) -- 4.2x fewer FLOPs.

Precision: fp16 operands (11-bit mantissa) with fp32 PSUM accumulation.
TensorE streams fp16 at 1 row/cycle (same rate as bf16/f32r, 4x fp32 mode).
Per-operand rounding is ~2^-12 relative; over the 2304-term conv reduction
the output error stays ~4e-4 relative -- far inside the 2e-2 gate -- while
using a third of the PE cycles of the fp32-class split-matmul scheme and
half the input DMA bytes.

Schedule: row-tiles of 9,9,9,9,9,8,3 rows per image (the 3-row tail keeps
the end-of-kernel mix+copy+DMA chain short).  PE is pre-warmed with dummy
matmuls (reading a not-yet-written x region, results to an unread PSUM
bank) so the clock is fully ramped and the tensor engine never idles
while the first weight/input DMAs are in flight.  Each tile's mix is
split into ob-pair halves interleaved into the next tile's conv, so
output copies/DMAs start half a tile early and stream out continuously.
Output is staged in SBUF as fp16 (host upcasts to fp32; ~1.4e-4 extra
RMS rounding) which halves the output DMA bytes and keeps the kernel
compute-bound.

Sharding: data-parallel over batch N=16 -> 2 images per core on 8 cores.
dictionary (as [128, 1800] fp16 tap matrices) and M^T are replicated.
"""

import numpy as np

N_CORES = 8
IMGS_PER_CORE = 2
CIN = 256
COUT = 512
NDICT = 100
H = W = 56
HP = WP = 58  # padded
S = 3  # lookup sparsity
HW = H * W  # 3136

# (h0, rows) conv tiles per image; free dim = rows*W <= 504 (PSUM bank cap)
TILES = [(0, 9), (9, 9), (18, 9), (27, 9), (36, 9), (45, 8), (53, 3)]
# padded-input row chunks; tile t only needs chunks 0..ceil
ROW_CHUNKS = [(0, 11), (11, 9), (20, 9), (29, 9), (38, 9), (47, 11)]
MAX_FREE = 9 * W  # 504

TRACE = False  # set by test.py to get a profile
_LAST_RESULTS = {}  # test.py reads exec_time_ns from here


def _build_program():
    import concourse.bacc as bacc
    import concourse.mybir as mybir
    import concourse.tile as tile

    f32 = mybir.dt.float32
    f16 = mybir.dt.float16

    nc = bacc.Bacc("TRN2", target_bir_lowering=False, debug=False)

    x_d = nc.dram_tensor("x", (IMGS_PER_CORE, CIN, HP, WP), f16,
                         kind="ExternalInput")
    w_d = nc.dram_tensor("w", (128, 2 * 9 * NDICT), f16, kind="ExternalInput")
    m_d = nc.dram_tensor("m", (NDICT, COUT), f16, kind="ExternalInput")
    out_d = nc.dram_tensor("out", (IMGS_PER_CORE, COUT, H, W), f16,
                           kind="ExternalOutput")

    with tile.TileContext(nc) as tc:
        with (
            tc.tile_pool(name="consts", bufs=1) as consts,
            tc.tile_pool(name="xpool", bufs=1) as xpool,
            tc.tile_pool(name="ypool", bufs=4) as ypool,
            tc.tile_pool(name="opool", bufs=2) as opool,
            tc.tile_pool(name="psum_y", bufs=2, space="PSUM") as psum_y_pool,
            tc.tile_pool(name="psum_o", bufs=6, space="PSUM") as psum_o_pool,
        ):
            # --- constants + input tiles
            w_sb = consts.tile([128, 2 * 9 * NDICT], f16)
            m_sb = consts.tile([NDICT, COUT], f16)
            x_sb = xpool.tile([128, IMGS_PER_CORE, 2, HP, WP], f16, tag="x_sb")
            x_v = x_d.rearrange("i (b c) h w -> c i b h w", c=128)

            # --- PE pre-warm: dummy matmuls keep the PE clock ramping while
            # the first weight/input DMAs are in flight.  They read the img1
            # region of x_sb, which is written only much later (WAR dep), so
            # they start immediately; results go to a scratch PSUM bank that
            # is never read.
            pwarm = psum_o_pool.tile([128, MAX_FREE], f32, tag="po",
                                     name="pwarm")
            for _ in range(33):
                nc.tensor.matmul(pwarm[:58, :116], x_sb[:, 1, 0, 0, 0:58],
                                 x_sb[:, 1, 0, 1:3, :],
                                 start=True, stop=True)

            def dma_x(img, cb, chunk):
                r0, nr = ROW_CHUNKS[chunk]
                nc.sync.dma_start(x_sb[:, img, cb, r0:r0 + nr, :],
                                  x_v[:, img, cb, r0:r0 + nr, :])

            nc.sync.dma_start(w_sb[:, :900], w_d[:, :900])
            dma_x(0, 0, 0)
            nc.sync.dma_start(w_sb[:, 900:], w_d[:, 900:])
            dma_x(0, 1, 0)
            nc.sync.dma_start(m_sb[:], m_d[:])
            for img in range(IMGS_PER_CORE):
                for chunk in range(1 if img == 0 else 0, len(ROW_CHUNKS)):
                    for cb in range(2):
                        if img == 0 and chunk == 0 and cb == 0:
                            continue
                        dma_x(img, cb, chunk)

            out_v = out_d.rearrange("i (b o) h w -> i b o (h w)", o=128)
            # same tensor viewed [img, o, b, hw] so a [128, 4, cols] SBUF
            # tile maps element-for-element in the tail DMA
            out_vt = out_d.rearrange("i (b o) h w -> i o b (h w)", o=128)

            # ti order [1, 0, 2]: the ti=1 taps always cover the full,
            # valid row range, so the group-opening (start=True) matmul
            # touches every PSUM element; edge taps of the image-border
            # tiles then skip the all-zero pad row (saves its cycles --
            # per-element has_written turns their first touch into a write)
            TAP_ORDER = [3, 4, 5, 0, 1, 2, 6, 7, 8]

            def emit_conv_taps(py, img, ti_idx, cb, t0, t1):
                h0, rows = TILES[ti_idx]
                fd = rows * W
                for tt in TAP_ORDER[t0:t1]:
                    ti, tj = divmod(tt, 3)
                    k = cb * 9 + tt
                    tap = slice(k * NDICT, (k + 1) * NDICT)
                    r_lo, r_hi, c_lo = 0, rows, 0
                    if ti == 0 and h0 == 0:
                        r_lo, c_lo = 1, W  # top pad row is zero: skip it
                    if ti == 2 and h0 + rows == H:
                        r_hi = rows - 1    # bottom pad row is zero: skip it
                    rh = (slice(None), img, cb,
                          slice(h0 + ti + r_lo, h0 + ti + r_hi),
                          slice(tj, tj + W))
                    nc.tensor.matmul(
                        py[:, c_lo:r_hi * W], w_sb[:, tap], x_sb[rh],
                        start=(cb == 0 and tt == TAP_ORDER[0]),
                        stop=(cb == 1 and tt == TAP_ORDER[-1]))

            o_accs = {}

            def emit_y_copy(py, img, ti_idx):
                _, rows = TILES[ti_idx]
                fd = rows * W
                if ti_idx == 0:
                    o_accs[img] = opool.tile([128, 4, HW], f16, tag="oacc",
                                             name=f"oacc{img}")
                y16 = ypool.tile([NDICT, MAX_FREE], f16, tag="y16")
                nc.scalar.copy(y16[:, :fd], py[:, :fd])
                return y16

            def emit_mix_half(y16, img, ti_idx, half):
                h0, rows = TILES[ti_idx]
                fd = rows * W
                c0 = h0 * W
                o_acc = o_accs[img]
                if fd <= 256:
                    # both obs of this half fit in one PSUM bank: 2 matmuls,
                    # then a single copy (fewer fixed costs on the end chain)
                    po = psum_o_pool.tile([128, MAX_FREE], f32, tag="po")
                    po2 = po.rearrange("p (b f) -> p b f", b=2)
                    for i, ob in enumerate((2 * half, 2 * half + 1)):
                        obs = slice(ob * 128, (ob + 1) * 128)
                        nc.tensor.matmul(po2[:, i, :fd], m_sb[:, obs],
                                         y16[:, :fd], start=True, stop=True)
                    dst = o_acc[:, 2 * half:2 * half + 2, c0:c0 + fd]
                    if half == 0:
                        nc.vector.tensor_copy(dst, po2[:, :, :fd])
                    else:
                        nc.scalar.copy(dst, po2[:, :, :fd])
                else:
                    for ob in (2 * half, 2 * half + 1):
                        obs = slice(ob * 128, (ob + 1) * 128)
                        po = psum_o_pool.tile([128, MAX_FREE], f32, tag="po")
                        nc.tensor.matmul(po[:, :fd], m_sb[:, obs],
                                         y16[:, :fd], start=True, stop=True)
                        dst = o_acc[:, ob, c0:c0 + fd]
                        if ob % 2 == 0:
                            nc.vector.tensor_copy(dst, po[:, :fd])
                        else:
                            nc.scalar.copy(dst, po[:, :fd])
                # per-ob-pair DMA: output streams out as soon as each half of
                # the tile's mix lands (no end-of-kernel backlog).  Small
                # tail tiles go as a single all-ob DMA (HWDGE slot latency
                # dominates their transfer time).
                if fd >= 300:
                    nc.sync.dma_start(
                        out_vt[img, :, 2 * half:2 * half + 2, c0:c0 + fd],
                        o_acc[:, 2 * half:2 * half + 2, c0:c0 + fd])
                elif half == 1:
                    nc.sync.dma_start(out_vt[img, :, :, c0:c0 + fd],
                                      o_acc[:, :, c0:c0 + fd])

            # software-pipeline by one tile, with tile i-1's mix matmuls
            # interleaved into the middle of tile i's conv so its output
            # copies/DMA start ~half a tile earlier
            prev = None
            for img in range(IMGS_PER_CORE):
                for t in range(len(TILES)):
                    py = psum_y_pool.tile([NDICT, MAX_FREE], f32, tag="py",
                                          name=f"py{img}_{t}")
                    emit_conv_taps(py, img, t, 0, 0, 9)
                    if prev is not None:
                        emit_mix_half(*prev, 0)
                    emit_conv_taps(py, img, t, 1, 0, 9)
                    y16 = emit_y_copy(py, img, t)
                    if prev is not None:
                        emit_mix_half(*prev, 1)
                    prev = (y16, img, t)
            emit_mix_half(*prev, 0)
            emit_mix_half(*prev, 1)

    nc.compile()
    return nc


_NC_CACHE = None


def kernel(x, dictionary, lookup_indices, lookup_coefficients):
    global _NC_CACHE
    from concourse import bass_utils

    x = np.asarray(x, dtype=np.float32)
    dictionary = np.asarray(dictionary, dtype=np.float32)
    idx = np.asarray(lookup_indices).astype(np.int64)
    coef = np.asarray(lookup_coefficients, dtype=np.float32)

    # M^T[d, o] = sum_s coeff[o, s] * [idx[o, s] == d]
    mt = np.zeros((NDICT, COUT), np.float32)
    np.add.at(mt, (idx.reshape(-1),
                   np.repeat(np.arange(COUT), S)), coef.reshape(-1))

    # wt[c_in_block, (cblk, ti, tj, d)] = dictionary[d, cblk*128+c, ti, tj]
    wt = np.ascontiguousarray(
        dictionary.reshape(NDICT, 2, 128, 3, 3).transpose(2, 1, 3, 4, 0)
    ).reshape(128, 2 * 9 * NDICT)

    xp = np.pad(x, ((0, 0), (0, 0), (1, 1), (1, 1)))
    xp = np.ascontiguousarray(
        xp.reshape(N_CORES, IMGS_PER_CORE, CIN, HP, WP)).astype(np.float16)
    wt16 = wt.astype(np.float16)
    mt16 = mt.astype(np.float16)

    if _NC_CACHE is None:
        _NC_CACHE = _build_program()
    nc = _NC_CACHE

    in_maps = [{"x": xp[i], "w": wt16, "m": mt16} for i in range(N_CORES)]
    try:
        res = bass_utils.run_bass_kernel_spmd(
            nc, in_maps, core_ids=list(range(N_CORES)), trace=TRACE)
    except ModuleNotFoundError:
        # no axon NTFF profile hook in this environment
        res = bass_utils.run_bass_kernel_spmd(
            nc, in_maps, core_ids=list(range(N_CORES)), trace=False)
    _LAST_RESULTS["res"] = res

    out = np.concatenate([r["out"] for r in res.results], axis=0)
    return out.reshape(16, COUT, H, W).astype(np.float32)


# revision 42
# speedup vs baseline: 1.0215x; 1.0091x over previous
"""LookupConv2d Trainium2 kernel.

Math: out = conv2d(x, W), W[o] = sum_s coeff[o,s] * dictionary[idx[o,s]].
Factorization: W = M @ D where M[o,d] = sum_{s: idx[o,s]=d} coeff[o,s] is a
(512, 100) scatter of the coefficients.  Then
    out = M @ conv2d(x, dictionary)
i.e. a 100-channel conv (23 GFLOP) followed by a 1x1 512x100 mix (5 GFLOP)
instead of a 512-channel conv (118 GFLOP) -- 4.2x fewer FLOPs.

Precision: fp16 operands (11-bit mantissa) with fp32 PSUM accumulation.
TensorE streams fp16 at 1 row/cycle (same rate as bf16/f32r, 4x fp32 mode).
Per-operand rounding is ~2^-12 relative; over the 2304-term conv reduction
the output error stays ~4e-4 relative -- far inside the 2e-2 gate -- while
using a third of the PE cycles of the fp32-class split-matmul scheme and
half the input DMA bytes.

Schedule: row-tiles of 9,9,9,9,9,8,3 rows per image (the 3-row tail keeps
the end-of-kernel mix+copy+DMA chain short).  PE is pre-warmed with dummy
matmuls (reading a not-yet-written x region, results to an unread PSUM
bank) so the clock is fully ramped and the tensor engine never idles
while the first weight/input DMAs are in flight.  Each tile's mix is
split into ob-pair halves interleaved into the next tile's conv, so
output copies/DMAs start half a tile early and stream out continuously.
Output is staged in SBUF as fp16 (host upcasts to fp32; ~1.4e-4 extra
RMS rounding) which halves the output DMA bytes and keeps the kernel
compute-bound.

Sharding: data-parallel over batch N=16 -> 2 images per core on 8 cores.
dictionary (as [128, 1800] fp16 tap matrices) and M^T are replicated.
"""

import numpy as np

N_CORES = 8
IMGS_PER_CORE = 2
CIN = 256
COUT = 512
NDICT = 100
H = W = 56
HP = WP = 58  # padded
S = 3  # lookup sparsity
HW = H * W  # 3136

# (h0, rows) conv tiles per image; free dim = rows*W <= 504 (PSUM bank cap)
TILES = [(0, 9), (9, 9), (18, 9), (27, 9), (36, 9), (45, 8), (53, 3)]
# padded-input row chunks; tile t only needs chunks 0..ceil
ROW_CHUNKS = [(0, 11), (11, 9), (20, 9), (29, 9), (38, 9), (47, 11)]
MAX_FREE = 9 * W  # 504

TRACE = False  # set by test.py to get a profile
_LAST_RESULTS = {}  # test.py reads exec_time_ns from here


def _build_program():
    import concourse.bacc as bacc
    import concourse.mybir as mybir
    import concourse.tile as tile

    f32 = mybir.dt.float32
    f16 = mybir.dt.float16

    nc = bacc.Bacc("TRN2", target_bir_lowering=False, debug=False)

    x_d = nc.dram_tensor("x", (IMGS_PER_CORE, CIN, HP, WP), f16,
                         kind="ExternalInput")
    w_d = nc.dram_tensor("w", (128, 2 * 9 * NDICT), f16, kind="ExternalInput")
    m_d = nc.dram_tensor("m", (NDICT, COUT), f16, kind="ExternalInput")
    out_d = nc.dram_tensor("out", (IMGS_PER_CORE, COUT, H, W), f16,
                           kind="ExternalOutput")

    with tile.TileContext(nc) as tc:
        with (
            tc.tile_pool(name="consts", bufs=1) as consts,
            tc.tile_pool(name="xpool", bufs=1) as xpool,
            tc.tile_pool(name="ypool", bufs=4) as ypool,
            tc.tile_pool(name="opool", bufs=2) as opool,
            tc.tile_pool(name="psum_y", bufs=2, space="PSUM") as psum_y_pool,
            tc.tile_pool(name="psum_o", bufs=6, space="PSUM") as psum_o_pool,
        ):
            # --- constants + input tiles
            w_sb = consts.tile([128, 2 * 9 * NDICT], f16)
            m_sb = consts.tile([NDICT, COUT], f16)
            x_sb = xpool.tile([128, IMGS_PER_CORE, 2, HP, WP], f16, tag="x_sb")
            x_v = x_d.rearrange("i (b c) h w -> c i b h w", c=128)

            # --- PE pre-warm: dummy matmuls keep the PE clock ramping while
            # the first weight/input DMAs are in flight.  They read the img1
            # region of x_sb, which is written only much later (WAR dep), so
            # they start immediately; results go to a scratch PSUM bank that
            # is never read.
            pwarm = psum_o_pool.tile([128, MAX_FREE], f32, tag="po",
                                     name="pwarm")
            for _ in range(33):
                nc.tensor.matmul(pwarm[:58, :116], x_sb[:, 1, 0, 0, 0:58],
                                 x_sb[:, 1, 0, 1:3, :],
                                 start=True, stop=True)

            def dma_x(img, cb, chunk):
                r0, nr = ROW_CHUNKS[chunk]
                nc.sync.dma_start(x_sb[:, img, cb, r0:r0 + nr, :],
                                  x_v[:, img, cb, r0:r0 + nr, :])

            nc.sync.dma_start(w_sb[:, :900], w_d[:, :900])
            dma_x(0, 0, 0)
            nc.sync.dma_start(w_sb[:, 900:], w_d[:, 900:])
            dma_x(0, 1, 0)
            nc.sync.dma_start(m_sb[:], m_d[:])
            for img in range(IMGS_PER_CORE):
                for chunk in range(1 if img == 0 else 0, len(ROW_CHUNKS)):
                    for cb in range(2):
                        if img == 0 and chunk == 0 and cb == 0:
                            continue
                        dma_x(img, cb, chunk)

            out_v = out_d.rearrange("i (b o) h w -> i b o (h w)", o=128)
            # same tensor viewed [img, o, b, hw] so a [128, 4, cols] SBUF
            # tile maps element-for-element in the tail DMA
            out_vt = out_d.rearrange("i (b o) h w -> i o b (h w)", o=128)

            # tap (ti=1, tj=1) first: it covers the full valid row and
            # column range, so the group-opening (start=True) matmul touches
            # every PSUM element.  All-zero padding rows (image-border tiles,
            # ti edge taps) and padding columns (tj edge taps) are then
            # skipped entirely -- their contribution is exactly zero and
    # per-element has_written turns each element's first touch into a
            # plain write, so the math is bit-identical at fewer cycles.
            TAP_ORDER = [4, 3, 5, 0, 1, 2, 6, 7, 8]

            def emit_conv_taps(py, img, ti_idx, cb, t0, t1):
                h0, rows = TILES[ti_idx]
                py3 = py.rearrange("p (r w) -> p r w", w=W)
                for tt in TAP_ORDER[t0:t1]:
                    ti, tj = divmod(tt, 3)
                    k = cb * 9 + tt
                    tap = slice(k * NDICT, (k + 1) * NDICT)
                    r_lo, r_hi = 0, rows
                    if ti == 0 and h0 == 0:
                        r_lo = 1           # top pad row is zero: skip it
                    if ti == 2 and h0 + rows == H:
                        r_hi = rows - 1    # bottom pad row is zero: skip it
                    c_lo = 1 if tj == 0 else 0   # left pad col is zero
                    c_hi = W - 1 if tj == 2 else W  # right pad col is zero
                    rh = (slice(None), img, cb,
                          slice(h0 + ti + r_lo, h0 + ti + r_hi),
                          slice(tj + c_lo, tj + c_hi))
                    nc.tensor.matmul(
                        py3[:, r_lo:r_hi, c_lo:c_hi], w_sb[:, tap], x_sb[rh],
                        start=(cb == 0 and tt == TAP_ORDER[0]),
                        stop=(cb == 1 and tt == TAP_ORDER[-1]))

            o_accs = {}

            def emit_y_copy(py, img, ti_idx):
                _, rows = TILES[ti_idx]
                fd = rows * W
                if ti_idx == 0:
                    o_accs[img] = opool.tile([128, 4, HW], f16, tag="oacc",
                                             name=f"oacc{img}")
                y16 = ypool.tile([NDICT, MAX_FREE], f16, tag="y16")
                nc.scalar.copy(y16[:, :fd], py[:, :fd])
                return y16

            def emit_mix_half(y16, img, ti_idx, half):
                h0, rows = TILES[ti_idx]
                fd = rows * W
                c0 = h0 * W
                o_acc = o_accs[img]
                if fd <= 256:
                    # both obs of this half fit in one PSUM bank: 2 matmuls,
                    # then a single copy (fewer fixed costs on the end chain)
                    po = psum_o_pool.tile([128, MAX_FREE], f32, tag="po")
                    po2 = po.rearrange("p (b f) -> p b f", b=2)
                    for i, ob in enumerate((2 * half, 2 * half + 1)):
                        obs = slice(ob * 128, (ob + 1) * 128)
                        nc.tensor.matmul(po2[:, i, :fd], m_sb[:, obs],
                                         y16[:, :fd], start=True, stop=True)
                    dst = o_acc[:, 2 * half:2 * half + 2, c0:c0 + fd]
                    if half == 0:
                        nc.vector.tensor_copy(dst, po2[:, :, :fd])
                    else:
                        nc.scalar.copy(dst, po2[:, :, :fd])
                else:
                    for ob in (2 * half, 2 * half + 1):
                        obs = slice(ob * 128, (ob + 1) * 128)
                        po = psum_o_pool.tile([128, MAX_FREE], f32, tag="po")
                        nc.tensor.matmul(po[:, :fd], m_sb[:, obs],
                                         y16[:, :fd], start=True, stop=True)
                        dst = o_acc[:, ob, c0:c0 + fd]
                        if ob % 2 == 0:
                            nc.vector.tensor_copy(dst, po[:, :fd])
                        else:
                            nc.scalar.copy(dst, po[:, :fd])
                # per-ob-pair DMA: output streams out as soon as each half of
                # the tile's mix lands (no end-of-kernel backlog).  Small
                # tail tiles go as a single all-ob DMA (HWDGE slot latency
                # dominates their transfer time).
                if fd >= 300:
                    nc.sync.dma_start(
                        out_vt[img, :, 2 * half:2 * half + 2, c0:c0 + fd],
                        o_acc[:, 2 * half:2 * half + 2, c0:c0 + fd])
                elif half == 1:
                    nc.sync.dma_start(out_vt[img, :, :, c0:c0 + fd],
                                      o_acc[:, :, c0:c0 + fd])

            # software-pipeline by one tile, with tile i-1's mix matmuls
            # interleaved into the middle of tile i's conv so its output
            # copies/DMA start ~half a tile earlier
            prev = None
            for img in range(IMGS_PER_CORE):
                for t in range(len(TILES)):
                    py = psum_y_pool.tile([NDICT, MAX_FREE], f32, tag="py",
                                          name=f"py{img}_{t}")
                    emit_conv_taps(py, img, t, 0, 0, 9)
                    if prev is not None:
                        emit_mix_half(*prev, 0)
                    emit_conv_taps(py, img, t, 1, 0, 9)
                    y16 = emit_y_copy(py, img, t)
                    if prev is not None:
                        emit_mix_half(*prev, 1)
                    prev = (y16, img, t)
            emit_mix_half(*prev, 0)
            emit_mix_half(*prev, 1)

    nc.compile()
    return nc


_NC_CACHE = None


def kernel(x, dictionary, lookup_indices, lookup_coefficients):
    global _NC_CACHE
    from concourse import bass_utils

    x = np.asarray(x, dtype=np.float32)
    dictionary = np.asarray(dictionary, dtype=np.float32)
    idx = np.asarray(lookup_indices).astype(np.int64)
    coef = np.asarray(lookup_coefficients, dtype=np.float32)

    # M^T[d, o] = sum_s coeff[o, s] * [idx[o, s] == d]
    mt = np.zeros((NDICT, COUT), np.float32)
    np.add.at(mt, (idx.reshape(-1),
                   np.repeat(np.arange(COUT), S)), coef.reshape(-1))

    # wt[c_in_block, (cblk, ti, tj, d)] = dictionary[d, cblk*128+c, ti, tj]
    wt = np.ascontiguousarray(
        dictionary.reshape(NDICT, 2, 128, 3, 3).transpose(2, 1, 3, 4, 0)
    ).reshape(128, 2 * 9 * NDICT)

    xp = np.pad(x, ((0, 0), (0, 0), (1, 1), (1, 1)))
    xp = np.ascontiguousarray(
        xp.reshape(N_CORES, IMGS_PER_CORE, CIN, HP, WP)).astype(np.float16)
    wt16 = wt.astype(np.float16)
    mt16 = mt.astype(np.float16)

    if _NC_CACHE is None:
        _NC_CACHE = _build_program()
    nc = _NC_CACHE

    in_maps = [{"x": xp[i], "w": wt16, "m": mt16} for i in range(N_CORES)]
    try:
        res = bass_utils.run_bass_kernel_spmd(
            nc, in_maps, core_ids=list(range(N_CORES)), trace=TRACE)
    except ModuleNotFoundError:
        # no axon NTFF profile hook in this environment
        res = bass_utils.run_bass_kernel_spmd(
            nc, in_maps, core_ids=list(range(N_CORES)), trace=False)
    _LAST_RESULTS["res"] = res

    out = np.concatenate([r["out"] for r in res.results], axis=0)
    return out.reshape(16, COUT, H, W).astype(np.float32)


# revision 43
# speedup vs baseline: 1.0222x; 1.0006x over previous
"""LookupConv2d Trainium2 kernel.

Math: out = conv2d(x, W), W[o] = sum_s coeff[o,s] * dictionary[idx[o,s]].
Factorization: W = M @ D where M[o,d] = sum_{s: idx[o,s]=d} coeff[o,s] is a
(512, 100) scatter of the coefficients.  Then
    out = M @ conv2d(x, dictionary)
i.e. a 100-channel conv (23 GFLOP) followed by a 1x1 512x100 mix (5 GFLOP)
instead of a 512-channel conv (118 GFLOP) -- 4.2x fewer FLOPs.

Precision: fp16 operands (11-bit mantissa) with fp32 PSUM accumulation.
TensorE streams fp16 at 1 row/cycle (same rate as bf16/f32r, 4x fp32 mode).
Per-operand rounding is ~2^-12 relative; over the 2304-term conv reduction
the output error stays ~4e-4 relative -- far inside the 2e-2 gate -- while
using a third of the PE cycles of the fp32-class split-matmul scheme and
half the input DMA bytes.

Schedule: row-tiles of 9,9,9,9,9,8,3 rows per image (the 3-row tail keeps
the end-of-kernel mix+copy+DMA chain short).  PE is pre-warmed with dummy
matmuls (reading a not-yet-written x region, results to an unread PSUM
bank) so the clock is fully ramped and the tensor engine never idles
while the first weight/input DMAs are in flight.  Each tile's mix is
split into ob-pair halves interleaved into the next tile's conv, so
output copies/DMAs start half a tile early and stream out continuously.
Output is staged in SBUF as fp16 (host upcasts to fp32; ~1.4e-4 extra
RMS rounding) which halves the output DMA bytes and keeps the kernel
compute-bound.

Sharding: data-parallel over batch N=16 -> 2 images per core on 8 cores.
dictionary (as [128, 1800] fp16 tap matrices) and M^T are replicated.
"""

import numpy as np

N_CORES = 8
IMGS_PER_CORE = 2
CIN = 256
COUT = 512
NDICT = 100
H = W = 56
HP = WP = 58  # padded
S = 3  # lookup sparsity
HW = H * W  # 3136

# (h0, rows) conv tiles per image; free dim = rows*W <= 504 (PSUM bank cap)
TILES = [(0, 9), (9, 9), (18, 9), (27, 9), (36, 9), (45, 8), (53, 3)]
# padded-input row chunks; tile t only needs chunks 0..ceil
ROW_CHUNKS = [(1, 10), (11, 9), (20, 9), (29, 9), (38, 9), (47, 10)]
MAX_FREE = 9 * W  # 504

TRACE = False  # set by test.py to get a profile
_LAST_RESULTS = {}  # test.py reads exec_time_ns from here


def _build_program():
    import concourse.bacc as bacc
    import concourse.mybir as mybir
    import concourse.tile as tile

    f32 = mybir.dt.float32
    f16 = mybir.dt.float16

    nc = bacc.Bacc("TRN2", target_bir_lowering=False, debug=False)

    x_d = nc.dram_tensor("x", (IMGS_PER_CORE, CIN, HP, WP), f16,
                         kind="ExternalInput")
    w_d = nc.dram_tensor("w", (128, 2 * 9 * NDICT), f16, kind="ExternalInput")
    m_d = nc.dram_tensor("m", (NDICT, COUT), f16, kind="ExternalInput")
    out_d = nc.dram_tensor("out", (IMGS_PER_CORE, COUT, H, W), f16,
                           kind="ExternalOutput")

    with tile.TileContext(nc) as tc:
        with (
            tc.tile_pool(name="consts", bufs=1) as consts,
            tc.tile_pool(name="xpool", bufs=1) as xpool,
            tc.tile_pool(name="ypool", bufs=4) as ypool,
            tc.tile_pool(name="opool", bufs=2) as opool,
            tc.tile_pool(name="psum_y", bufs=2, space="PSUM") as psum_y_pool,
            tc.tile_pool(name="psum_o", bufs=6, space="PSUM") as psum_o_pool,
        ):
            # --- constants + input tiles
            w_sb = consts.tile([128, 2 * 9 * NDICT], f16)
            m_sb = consts.tile([NDICT, COUT], f16)
            x_sb = xpool.tile([128, IMGS_PER_CORE, 2, HP, WP], f16, tag="x_sb")
            x_v = x_d.rearrange("i (b c) h w -> c i b h w", c=128)

            # --- PE pre-warm: dummy matmuls keep the PE clock ramping while
            # the first weight/input DMAs are in flight.  They read the img1
            # region of x_sb, which is written only much later (WAR dep), so
            # they start immediately; results go to a scratch PSUM bank that
            # is never read.
            pwarm = psum_o_pool.tile([128, MAX_FREE], f32, tag="po",
                                     name="pwarm")
            for _ in range(33):
                nc.tensor.matmul(pwarm[:58, :116], x_sb[:, 1, 0, 0, 0:58],
                                 x_sb[:, 1, 0, 1:3, :],
                                 start=True, stop=True)

            def dma_x(img, cb, chunk):
                r0, nr = ROW_CHUNKS[chunk]
                nc.sync.dma_start(x_sb[:, img, cb, r0:r0 + nr, :],
                                  x_v[:, img, cb, r0:r0 + nr, :])

            nc.sync.dma_start(w_sb[:, :900], w_d[:, :900])
            dma_x(0, 0, 0)
            nc.sync.dma_start(w_sb[:, 900:], w_d[:, 900:])
            dma_x(0, 1, 0)
            nc.sync.dma_start(m_sb[:], m_d[:])
            for img in range(IMGS_PER_CORE):
                for chunk in range(1 if img == 0 else 0, len(ROW_CHUNKS)):
                    for cb in range(2):
                        if img == 0 and chunk == 0 and cb == 0:
                            continue
                        dma_x(img, cb, chunk)

            out_v = out_d.rearrange("i (b o) h w -> i b o (h w)", o=128)
            # same tensor viewed [img, o, b, hw] so a [128, 4, cols] SBUF
            # tile maps element-for-element in the tail DMA
            out_vt = out_d.rearrange("i (b o) h w -> i o b (h w)", o=128)

            # tap (ti=1, tj=1) first: it covers the full valid row and
            # column range, so the group-opening (start=True) matmul touches
            # every PSUM element.  All-zero padding rows (image-border tiles,
            # ti edge taps) and padding columns (tj edge taps) are then
            # skipped entirely -- their contribution is exactly zero and
    # per-element has_written turns each element's first touch into a
            # plain write, so the math is bit-identical at fewer cycles.
            TAP_ORDER = [4, 3, 5, 0, 1, 2, 6, 7, 8]

            def emit_conv_taps(py, img, ti_idx, cb, t0, t1):
                h0, rows = TILES[ti_idx]
                py3 = py.rearrange("p (r w) -> p r w", w=W)
                for tt in TAP_ORDER[t0:t1]:
                    ti, tj = divmod(tt, 3)
                    k = cb * 9 + tt
                    tap = slice(k * NDICT, (k + 1) * NDICT)
                    r_lo, r_hi = 0, rows
                    if ti == 0 and h0 == 0:
                        r_lo = 1           # top pad row is zero: skip it
                    if ti == 2 and h0 + rows == H:
                        r_hi = rows - 1    # bottom pad row is zero: skip it
                    c_lo = 1 if tj == 0 else 0   # left pad col is zero
                    c_hi = W - 1 if tj == 2 else W  # right pad col is zero
                    rh = (slice(None), img, cb,
                          slice(h0 + ti + r_lo, h0 + ti + r_hi),
                          slice(tj + c_lo, tj + c_hi))
                    nc.tensor.matmul(
                        py3[:, r_lo:r_hi, c_lo:c_hi], w_sb[:, tap], x_sb[rh],
                        start=(cb == 0 and tt == TAP_ORDER[0]),
                        stop=(cb == 1 and tt == TAP_ORDER[-1]))

            o_accs = {}

            def emit_y_copy(py, img, ti_idx):
                _, rows = TILES[ti_idx]
                fd = rows * W
                if ti_idx == 0:
                    o_accs[img] = opool.tile([128, 4, HW], f16, tag="oacc",
                                             name=f"oacc{img}")
                y16 = ypool.tile([NDICT, MAX_FREE], f16, tag="y16")
                nc.scalar.copy(y16[:, :fd], py[:, :fd])
                return y16

            def emit_mix_half(y16, img, ti_idx, half):
                h0, rows = TILES[ti_idx]
                fd = rows * W
                c0 = h0 * W
                o_acc = o_accs[img]
                if fd <= 256:
                    # both obs of this half fit in one PSUM bank: 2 matmuls,
                    # then a single copy (fewer fixed costs on the end chain)
                    po = psum_o_pool.tile([128, MAX_FREE], f32, tag="po")
                    po2 = po.rearrange("p (b f) -> p b f", b=2)
                    for i, ob in enumerate((2 * half, 2 * half + 1)):
                        obs = slice(ob * 128, (ob + 1) * 128)
                        nc.tensor.matmul(po2[:, i, :fd], m_sb[:, obs],
                                         y16[:, :fd], start=True, stop=True)
                    dst = o_acc[:, 2 * half:2 * half + 2, c0:c0 + fd]
                    if half == 0:
                        nc.vector.tensor_copy(dst, po2[:, :, :fd])
                    else:
                        nc.scalar.copy(dst, po2[:, :, :fd])
                else:
                    for ob in (2 * half, 2 * half + 1):
                        obs = slice(ob * 128, (ob + 1) * 128)
                        po = psum_o_pool.tile([128, MAX_FREE], f32, tag="po")
                        nc.tensor.matmul(po[:, :fd], m_sb[:, obs],
                                         y16[:, :fd], start=True, stop=True)
                        dst = o_acc[:, ob, c0:c0 + fd]
                        if ob % 2 == 0:
                            nc.vector.tensor_copy(dst, po[:, :fd])
                        else:
                            nc.scalar.copy(dst, po[:, :fd])
                # per-ob-pair DMA: output streams out as soon as each half of
                # the tile's mix lands (no end-of-kernel backlog).  Small
                # tail tiles go as a single all-ob DMA (HWDGE slot latency
                # dominates their transfer time).
                if fd >= 300:
                    nc.sync.dma_start(
                        out_vt[img, :, 2 * half:2 * half + 2, c0:c0 + fd],
                        o_acc[:, 2 * half:2 * half + 2, c0:c0 + fd])
                elif half == 1:
                    nc.sync.dma_start(out_vt[img, :, :, c0:c0 + fd],
                                      o_acc[:, :, c0:c0 + fd])

            # software-pipeline by one tile, with tile i-1's mix matmuls
            # interleaved into the middle of tile i's conv so its output
            # copies/DMA start ~half a tile earlier
            prev = None
            for img in range(IMGS_PER_CORE):
                for t in range(len(TILES)):
                    py = psum_y_pool.tile([NDICT, MAX_FREE], f32, tag="py",
                                          name=f"py{img}_{t}")
                    emit_conv_taps(py, img, t, 0, 0, 9)
                    if prev is not None:
                        emit_mix_half(*prev, 0)
                    emit_conv_taps(py, img, t, 1, 0, 9)
                    y16 = emit_y_copy(py, img, t)
                    if prev is not None:
                        emit_mix_half(*prev, 1)
                    prev = (y16, img, t)
            emit_mix_half(*prev, 0)
            emit_mix_half(*prev, 1)

    nc.compile()
    return nc


_NC_CACHE = None


def kernel(x, dictionary, lookup_indices, lookup_coefficients):
    global _NC_CACHE
    from concourse import bass_utils

    x = np.asarray(x, dtype=np.float32)
    dictionary = np.asarray(dictionary, dtype=np.float32)
    idx = np.asarray(lookup_indices).astype(np.int64)
    coef = np.asarray(lookup_coefficients, dtype=np.float32)

    # M^T[d, o] = sum_s coeff[o, s] * [idx[o, s] == d]
    mt = np.zeros((NDICT, COUT), np.float32)
    np.add.at(mt, (idx.reshape(-1),
                   np.repeat(np.arange(COUT), S)), coef.reshape(-1))

    # wt[c_in_block, (cblk, ti, tj, d)] = dictionary[d, cblk*128+c, ti, tj]
    wt = np.ascontiguousarray(
        dictionary.reshape(NDICT, 2, 128, 3, 3).transpose(2, 1, 3, 4, 0)
    ).reshape(128, 2 * 9 * NDICT)

    xp = np.pad(x, ((0, 0), (0, 0), (1, 1), (1, 1)))
    xp = np.ascontiguousarray(
        xp.reshape(N_CORES, IMGS_PER_CORE, CIN, HP, WP)).astype(np.float16)
    wt16 = wt.astype(np.float16)
    mt16 = mt.astype(np.float16)

    if _NC_CACHE is None:
        _NC_CACHE = _build_program()
    nc = _NC_CACHE

    in_maps = [{"x": xp[i], "w": wt16, "m": mt16} for i in range(N_CORES)]
    try:
        res = bass_utils.run_bass_kernel_spmd(
            nc, in_maps, core_ids=list(range(N_CORES)), trace=TRACE)
    except ModuleNotFoundError:
        # no axon NTFF profile hook in this environment
        res = bass_utils.run_bass_kernel_spmd(
            nc, in_maps, core_ids=list(range(N_CORES)), trace=False)
    _LAST_RESULTS["res"] = res

    out = np.concatenate([r["out"] for r in res.results], axis=0)
    return out.reshape(16, COUT, H, W).astype(np.float32)
